# revision 21
# baseline (speedup 1.0000x reference)
"""Trainium2 Bass kernel for bidirectional gated linear recurrence block, v7.

v7 = v6 + deeper SBUF tile pools (xin 4, h1p 6, ew 5, yout 3 -- extra
scheduling slack at zero sim cost) and a `repeat` build parameter that
emits the body N times in one NEFF so the benchmark can amortize the
multi-hundred-microsecond per-launch host/tunnel dispatch cost out of
the per-exec device-time measurement. Cost-model engine occupancy per
exec: ACT 74us (critical, ~100%% busy in steady state), PE 68us, DVE
41us, DMA 39us; CoreSim end-to-end 79.7us (71.4us/rep at repeat=24 --
cross-rep pipelining hides warmup/drain), HW-measured ~70.1-70.6us/exec
(128x-amortized lower-quartile slope). Rejected by experiment: zf-first ACT
order (+0.5us), pair-merged h1 act on single-buffered 4-bank pH
(+32us -- L1 stalls on the pair act), SBUF-bf16 gate for the scan
(+0.9us -- ACT pays SBUF access on the F act).


Reference computation (per spatial position, channel-mixing MLPs):
  Z = tanh(W_z2 @ tanh(W_z1 @ x + b_z1) + b_z2)
  F = sigmoid(W_f2 @ tanh(W_f1 @ x + b_f1) + b_f2)
  channels 0:32  : h_t = F*h_{t-1} + (1-F)*Z forward over T
  channels 32:64 : same recurrence backward over T

Sharding: H (=64) split across 8 cores, 8 rows each; no collectives.

v6 = v5 + warmup/drain trims: per-h-pair x/y DMAs, segment-start memset
on DVE, and a dummy sigmoid up front so the combined tanh+sigmoid ACT
table loads once during warmup.

v5 core: explicit software pipelining. Per 512-position block k the work is
  PE : L1(k) 2 matmuls (427ns) + L2(k) 8 narrow matmuls (1707ns)
  ACT: h1(k) tanh [128,1024] (997ns) + z(k)/F(k) [128,512] (612ns each)
and the two engines have near-equal totals (~2.2us/block), so the emit
order skews L1 one block ahead and orders L2 z-chain first:
  PE queue : L1(k), L2(k-1)   |  ACT queue: z(k-1), F(k-1), h1(k)
which lets each engine run back-to-back instead of ping-ponging.

Other structure (from v4):
  - bf16 input [B, HH, C, 2, T, 2, W]; one 16KB-row DMA per h-quad.
  - L2 bwd-channel matmuls read h1 through time-REVERSED rhs APs so all
    post-L2 stages are direction-uniform: ONE 128-partition scan per pair.
    Host un-reverses bwd time at unshard.
  - z weights/bias negated -> ACT emits -z; g = (F-1)(-z) = (1-F)z in one
    scalar_tensor_tensor. Gate a = F from Sigmoid (same ACT table as Tanh);
    one memset zeroes segment starts.
  - PSUM: pH x2 + pZ x2 + pF x2 = 8 banks, all double-buffered.
  - bf16 output in scan layout; one DMA per h-quad on the Pool queue.
"""

import numpy as np

B, C, T, H, W = 2, 64, 32, 64, 64
NCORES = 8
HL = H // NCORES          # 8 h-rows per core
HP = HL // 2              # 4 h-pair tiles per batch entry
HH = HP // 2              # 2 h-quads per batch entry
S = 2 * W                 # 128 positions per h-pair (h2, w)
NJ = 4                    # 4 block-pairs per h-pair
CH = C // 2               # 32 = fwd (or bwd) channel count

_built = {}


def _build(repeat=1, order="h1_first", h1pair=False, quad_dma=False):
    """Build the kernel. repeat>1 emits the whole body that many times
    back-to-back in one NEFF (used by the benchmark to amortize per-launch
    host/runtime overhead out of the per-exec device-time measurement; each
    repeat re-does the full x-load/compute/y-store). order selects the
    ACT-queue emission order per pipeline iteration:
      "h1_first": h1(k) then z(k-1), F(k-1)
      "zf_first": z(k-1), F(k-1) then h1(k)"""
    import concourse.bass as bass
    import concourse.mybir as mybir
    import concourse.tile as tile
    from concourse import bacc

    fp32 = mybir.dt.float32
    bf16 = mybir.dt.bfloat16
    nc = bacc.Bacc(None, target_bir_lowering=False)

    x = nc.dram_tensor(
        "x", [B, HH, C, 2, NJ, 2, 16, T], bf16, kind="ExternalInput"
    )
    wall = nc.dram_tensor(
        "wall", [3 * C + 2, 2 * C], bf16, kind="ExternalInput"
    )
    y = nc.dram_tensor(
        "y", [B, HH, 2 * C, 2, NJ * 512], bf16, kind="ExternalOutput"
    )

    MUL = mybir.AluOpType.mult
    ADD = mybir.AluOpType.add
    SUB = mybir.AluOpType.subtract
    TANH = mybir.ActivationFunctionType.Tanh
    SIGM = mybir.ActivationFunctionType.Sigmoid

    def rev_t(ap2d, tlen):
        # [P, F] contiguous AP -> [P, seg, t] with innermost t reversed.
        (pstep, pcnt), (fstep, fcnt) = [list(d) for d in ap2d.ap]
        assert fstep == 1 and fcnt % tlen == 0, ap2d.ap
        return bass.AP(
            tensor=ap2d.tensor,
            offset=ap2d.offset + (tlen - 1),
            ap=[[pstep, pcnt], [tlen, fcnt // tlen], [-1, tlen]],
        )

    with tile.TileContext(nc) as tc:
        with (
            tc.tile_pool(name="consts", bufs=1) as consts,
            tc.tile_pool(name="xin", bufs=4) as xin,
            tc.tile_pool(name="h1p", bufs=6) as h1p,
            tc.tile_pool(name="ew", bufs=5) as ew,
            tc.tile_pool(name="yout", bufs=3) as yout,
            tc.tile_pool(name="psH", bufs=(1 if h1pair else 2),
                         space="PSUM") as psH,
            tc.tile_pool(name="psZF", bufs=2, space="PSUM") as psZF,
        ):
            # consts first on SP: tiny transfers, done before the first
            # (small, per-block) x slice lands.
            w1_sb = consts.tile([C, 2 * C], bf16)
            w2b_sb = consts.tile([2 * C, C + 3], bf16)
            nc.sync.dma_start(out=w1_sb, in_=wall[0:C, :])
            nc.sync.dma_start(out=w2b_sb, in_=wall[C : 3 * C, 0 : C + 3])
            w2_sb = w2b_sb[:, 0:C]
            # dummy sigmoid pins the tanh+sigmoid table load into warmup;
            # fed from a memset tile so it launches immediately.
            warm = consts.tile([2 * C, 2], bf16)
            nc.vector.memset(warm[:, 0:1], 0.0)
            nc.scalar.activation(warm[:, 1:2], warm[:, 0:1], SIGM)
            b_sb = consts.tile([2 * C, 3], fp32)
            nc.vector.tensor_copy(b_sb[:, :], w2b_sb[:, C : C + 3])
            # L2 bias vectors as matmul lhsT rows + a ones rhs row: biases
            # are injected into PSUM by K=1 accumulate-matmuls so the z/F
            # activations can merge into one bias-free tanh.
            blz_sb = consts.tile([1, 2 * C], bf16)
            blf_sb = consts.tile([1, 2 * C], bf16)
            nc.sync.dma_start(out=blz_sb, in_=wall[3 * C : 3 * C + 1, :])
            nc.sync.dma_start(
                out=blf_sb, in_=wall[3 * C + 1 : 3 * C + 2, :]
            )
            ones_sb = consts.tile([1, 512], bf16)
            nc.vector.memset(ones_sb[:, :], 1.0)

            # flat block list: (b, hh, u, jj, j2); 512 positions each
            blocks = [
                (b, hh, u, jj, j2)
                for b in range(B)
                for hh in range(HH)
                for u in range(2)
                for jj in range(NJ // 2)
                for j2 in range(2)
            ]
            N = len(blocks)
            quads = {}   # (b, hh, u) -> dict(x_t, y_t)
            st = {}      # k -> per-block tiles
            pairs = {}   # (b, hh, u, jj) -> dict(z, a, g)

            def emit_L1(k):
                b, hh, u, jj, j2 = blocks[k]
                qd = quads.get((b, hh, u))
                if qd is None:
                    x_t = xin.tile([C, NJ, 2, 16, T], bf16, name="x_t")
                    y_t = yout.tile([2 * C, NJ, 512], bf16, tag="yt", name="y_t")
                    qd = quads[(b, hh, u)] = dict(x_t=x_t, y_t=y_t)
                j = 2 * jj + j2
                if quad_dma:
                    # one [64 rows, 8KB] DMA per h-quad at its first block
                    if j == 0:
                        nc.sync.dma_start(
                            out=qd["x_t"].rearrange("c j r w t -> c (j r w t)"),
                            in_=x[b, hh, :, u].rearrange(
                                "c j r w t -> c (j r w t)"),
                        )
                else:
                    # per-block x slice: [64 rows, 2KB] contiguous both sides
                    nc.sync.dma_start(
                        out=qd["x_t"][:, j].rearrange("c r w t -> c (r w t)"),
                        in_=x[b, hh, :, u, j].rearrange("c r w t -> c (r w t)"),
                    )
                if h1pair:
                    if j2 == 0:
                        pH = psH.tile([2 * C, 4, 512], fp32)
                        st[k] = dict(pH=pH)
                    else:
                        pH = st[k - 1]["pH"]
                        st[k] = dict(pHref=pH)
                else:
                    pH = psH.tile([2 * C, 2, 512], fp32)
                    st[k] = dict(pH=pH)
                for q in range(2):
                    rhs = qd["x_t"][:, j, q]
                    nc.tensor.matmul(
                        pH[:, 2 * j2 + q if h1pair else q, :],
                        w1_sb[:, :], rhs, start=True, stop=True,
                    )

            def emit_h1(k):
                if h1pair:
                    # one [128, 2048] tanh per block-pair, fired at j2 == 1
                    b, hh, u, jj, j2 = blocks[k]
                    if j2 == 0:
                        return
                    pH = st[k].pop("pHref")
                    st[k - 1].pop("pH")
                    h1 = h1p.tile([2 * C, 4, 512], bf16)
                    nc.scalar.activation(
                        h1.rearrange("p a n -> p (a n)"),
                        pH.rearrange("p a n -> p (a n)"),
                        TANH, bias=b_sb[:, 0:1],
                    )
                    st[k - 1]["h1"] = h1[:, 0:2]
                    st[k]["h1"] = h1[:, 2:4]
                    return
                pH = st[k].pop("pH")
                h1 = h1p.tile([2 * C, 2, 512], bf16)
                nc.scalar.activation(
                    h1.rearrange("p a n -> p (a n)"),
                    pH.rearrange("p a n -> p (a n)"),
                    TANH, bias=b_sb[:, 0:1],
                )
                st[k]["h1"] = h1

            def emit_L2(k):
                h1 = st[k].pop("h1")
                pZF = psZF.tile([2 * C, 2, 512], fp32)
                # K=1 bias mms initialize both banks (start=True), then the
                # narrow channel mms accumulate on top (start=False).
                nc.tensor.matmul(
                    pZF[:, 0, :], blz_sb[0:1, :], ones_sb[0:1, :],
                    start=True, stop=False, tile_position=(0, 0),
                    skip_group_check=True,
                )
                nc.tensor.matmul(
                    pZF[:, 1, :], blf_sb[0:1, :], ones_sb[0:1, :],
                    start=True, stop=False, tile_position=(0, 0),
                    skip_group_check=True,
                )
                # z-chain first so the merged activation unblocks earlier
                for q in range(2):
                    col = 64 * q
                    hz = h1[0:C, q, :]
                    nc.tensor.matmul(
                        pZF[col : col + CH, 0, :], w2_sb[0:C, 0:CH], hz,
                        start=False, stop=True, tile_position=(0, col),
                        skip_group_check=True,
                    )
                    nc.tensor.matmul(
                        pZF[col + CH : col + C, 0, :], w2_sb[0:C, CH:C],
                        rev_t(hz, T),
                        start=False, stop=True, tile_position=(0, col + CH),
                        skip_group_check=True,
                    )
                for q in range(2):
                    col = 64 * q
                    hf = h1[C : 2 * C, q, :]
                    nc.tensor.matmul(
                        pZF[col : col + CH, 1, :], w2_sb[C : 2 * C, 0:CH], hf,
                        start=False, stop=True, tile_position=(64, col),
                        skip_group_check=True,
                    )
                    nc.tensor.matmul(
                        pZF[col + CH : col + C, 1, :], w2_sb[C : 2 * C, CH:C],
                        rev_t(hf, T),
                        start=False, stop=True, tile_position=(64, col + CH),
                        skip_group_check=True,
                    )
                st[k]["pZF"] = pZF

            def emit_zf(k):
                b, hh, u, jj, j2 = blocks[k]
                pZF = st[k].pop("pZF")
                zv = ew.tile([2 * C, 2, 512], bf16, tag="zv", name="zv_sb")
                # merged z+F activation: one bias-free tanh over both PSUM
                # banks. Bank 0 holds -z (negated wz2/bz2); bank 1 holds
                # v = tanh((wf2 h + bf2)/2) via 0.5-scaled weights, so
                # F = 0.5 + 0.5 v (computed on DVE in emit_epi).
                nc.scalar.activation(
                    zv.rearrange("p a n -> p (a n)"),
                    pZF.rearrange("p a n -> p (a n)"), TANH,
                )
                st[k]["zv"] = zv

            def emit_epi(k, final=False):
                # per-block gate+scan from the merged act output zv:
                #   zbar = zv[:,0] = -z,  v = zv[:,1] = tanh(u/2)
                #   gate a = 0.5 + 0.5 v = sigmoid(u)
                #   gtil  = (v - 1)*zbar = (1-v)*z = 2*(1-F)*z
                # scan state is 2h (a*2h + 2g); host halves at unshard.
                b, hh, u, jj, j2 = blocks[k]
                zv = st[k].pop("zv")
                del st[k]
                qd = quads[(b, hh, u)]
                zbar = zv[:, 0, :]
                v = zv[:, 1, :]
                g_t = ew.tile([2 * C, 512], bf16, tag="g", name="g_sb")
                a_t = ew.tile([2 * C, 512], bf16, tag="a", name="a_sb")
                nc.vector.scalar_tensor_tensor(g_t[:, :], v, 1.0, zbar, SUB, MUL)
                nc.vector.tensor_scalar(a_t[:, :], v, 0.5, 0.5, MUL, ADD)
                a3 = a_t.rearrange("p (s t) -> p s t", s=16)
                nc.vector.memset(a3[:, :, 0], 0.0)
                ysl = slice(2 * jj + j2, 2 * jj + j2 + 1)
                yj = qd["y_t"][:, ysl].rearrange("p j n -> p (j n)")
                nc.vector.tensor_tensor_scan(
                    yj, a_t[:, :], g_t[:, :], 0.0, MUL, ADD
                )
                if j2 != 1:
                    return
                last_tile = (b, hh, u) == blocks[-1][:3]
                if last_tile and jj == 0:
                    # last tile: ship the first half as soon as it's done
                    nc.gpsimd.dma_start(
                        out=y[b, hh, :, u, 0 : NJ * 256],
                        in_=qd["y_t"][:, 0 : NJ // 2].rearrange(
                            "p j n -> p (j n)"
                        ),
                    )
                elif last_tile and jj == NJ // 2 - 1:
                    # ship the last pair per block: the j=2 slice overlaps
                    # the final block's gate+scan; the tail DMA is half-size
                    y_t = qd.pop("y_t")
                    nc.gpsimd.dma_start(
                        out=y[b, hh, :, u, NJ * 256 : NJ * 384],
                        in_=y_t[:, 2:3].rearrange("p j n -> p (j n)"),
                    )
                    nc.gpsimd.dma_start(
                        out=y[b, hh, :, u, NJ * 384 :],
                        in_=y_t[:, 3:4].rearrange("p j n -> p (j n)"),
                    )
                    del quads[(b, hh, u)]
                elif jj == NJ // 2 - 1:
                    nc.gpsimd.dma_start(
                        out=y[b, hh, :, u],
                        in_=qd.pop("y_t").rearrange("p j n -> p (j n)"),
                    )
                    del quads[(b, hh, u)]

            # software-pipelined emission (skew 1):
            #   PE : L1(k), L2(k-1) | ACT: z(k-1), F(k-1), h1(k)
            for _rep in range(repeat):
                emit_L1(0)
                emit_h1(0)
                for k in range(1, N):
                    emit_L1(k)
                    if h1pair:
                        # pair-act must precede L2 of the pair's 1st block
                        emit_h1(k)
                        emit_L2(k - 1)
                        emit_zf(k - 1)
                    elif order == "zf_first":
                        emit_L2(k - 1)
                        emit_zf(k - 1)
                        emit_h1(k)
                    else:
                        emit_L2(k - 1)
                        emit_h1(k)
                        emit_zf(k - 1)
                    emit_epi(k - 1, final=(k - 1 == N - 2))
                emit_L2(N - 1)
                emit_zf(N - 1)
                emit_epi(N - 1, final=True)
    nc.compile()
    return nc


def _prep_weights(wz1, bz1, wz2, bz2, wf1, bf1, wf2, bf2):
    import ml_dtypes

    f32 = np.float32
    wall = np.zeros((3 * C + 2, 2 * C), dtype=f32)
    wall[0:C, :] = np.concatenate([wz1, wf1], axis=0).T  # L1 lhsT
    wall[C : 2 * C, 0:C] = -wz2.T                        # negated: tanh -> -z
    wall[2 * C : 3 * C, 0:C] = 0.5 * wf2.T               # sigmoid-as-tanh
    # biases ride in an unused corner of wall (bf16; converted on-device)
    wall[C : 3 * C, C : C + 3] = np.stack(
        [
            np.concatenate([bz1, bf1]),
            np.concatenate([-bz2, -bz2]),
            np.concatenate([bf2, bf2]),
        ],
        axis=1,
    )
    # L2 bias vectors as K=1 matmul lhsT rows (accumulated into PSUM)
    wall[3 * C, :] = np.concatenate([-bz2, -bz2])
    wall[3 * C + 1, :] = 0.5 * np.concatenate([bf2, bf2])
    wall = wall.astype(ml_dtypes.bfloat16)
    return dict(wall=wall)


def _prep_x(xin):
    """Full fp32 input -> per-core bf16 [B, HH, C, u, j, r, w16, T] shards:
    each 512-position block is a 2KB-contiguous run per channel row."""
    import ml_dtypes

    xb = np.asarray(xin, dtype=np.float32).astype(ml_dtypes.bfloat16)
    xr = xb.reshape(B, C, T, NCORES, HH, 2, 2, NJ, 16)
    return [
        np.ascontiguousarray(
            xr[:, :, :, core].transpose(0, 3, 1, 4, 6, 5, 7, 2)
        )
        for core in range(NCORES)
    ]


def _unshard_y(results):
    """Per-core bf16 [B, HH, 128, 2, 2048] -> full fp32 [B, C, T, H, W].

    Row r = 64q + 32d + c (q = h-row in pair, d = direction); free
    (u = h-pair in quad, j, s, t); bwd rows (d=1) carry time reversed.
    """
    outs = []
    for r in results:
        buf = (np.asarray(r["y"], dtype=np.float32) * 0.5).reshape(
            B, HH, 2, 2, CH, 2, NJ, 16, T
        )
        fwd = buf[:, :, :, 0]
        bwd = buf[:, :, :, 1, :, :, :, :, ::-1]
        st = np.stack([fwd, bwd], axis=3)  # [B, HH, q, d, c, u, j, s, t]
        o = st.transpose(0, 3, 4, 8, 1, 5, 2, 6, 7).reshape(B, C, T, HL, W)
        outs.append(o)
    return np.concatenate(outs, axis=3)


def kernel(inputs, wz1, bz1, wz2, bz2, wf1, bf1, wf2, bf2):
    from concourse.bass_utils import run_bass_kernel_spmd

    if "nc" not in _built:
        _built["nc"] = _build()
    nc = _built["nc"]

    wd = _prep_weights(
        np.asarray(wz1), np.asarray(bz1), np.asarray(wz2), np.asarray(bz2),
        np.asarray(wf1), np.asarray(bf1), np.asarray(wf2), np.asarray(bf2),
    )
    in_maps = []
    for shard in _prep_x(inputs):
        m = {"x": shard}
        m.update(wd)
        in_maps.append(m)

    res = run_bass_kernel_spmd(nc, in_maps, core_ids=list(range(NCORES)))
    return _unshard_y(res.results)



# revision 22
# speedup vs baseline: 1.1257x; 1.1257x over previous
"""Trainium2 Bass kernel for bidirectional gated linear recurrence block, v7.

v7 = v6 + deeper SBUF tile pools (xin 4, h1p 6, ew 5, yout 3 -- extra
scheduling slack at zero sim cost) and a `repeat` build parameter that
emits the body N times in one NEFF so the benchmark can amortize the
multi-hundred-microsecond per-launch host/tunnel dispatch cost out of
the per-exec device-time measurement. Cost-model engine occupancy per
exec: ACT 74us (critical, ~100%% busy in steady state), PE 68us, DVE
41us, DMA 39us; CoreSim end-to-end 79.7us (71.4us/rep at repeat=24 --
cross-rep pipelining hides warmup/drain), HW-measured ~70.1-70.6us/exec
(128x-amortized lower-quartile slope). Rejected by experiment: zf-first ACT
order (+0.5us), pair-merged h1 act on single-buffered 4-bank pH
(+32us -- L1 stalls on the pair act), SBUF-bf16 gate for the scan
(+0.9us -- ACT pays SBUF access on the F act).


Reference computation (per spatial position, channel-mixing MLPs):
  Z = tanh(W_z2 @ tanh(W_z1 @ x + b_z1) + b_z2)
  F = sigmoid(W_f2 @ tanh(W_f1 @ x + b_f1) + b_f2)
  channels 0:32  : h_t = F*h_{t-1} + (1-F)*Z forward over T
  channels 32:64 : same recurrence backward over T

Sharding: H (=64) split across 8 cores, 8 rows each; no collectives.

v6 = v5 + warmup/drain trims: per-h-pair x/y DMAs, segment-start memset
on DVE, and a dummy sigmoid up front so the combined tanh+sigmoid ACT
table loads once during warmup.

v5 core: explicit software pipelining. Per 512-position block k the work is
  PE : L1(k) 2 matmuls (427ns) + L2(k) 8 narrow matmuls (1707ns)
  ACT: h1(k) tanh [128,1024] (997ns) + z(k)/F(k) [128,512] (612ns each)
and the two engines have near-equal totals (~2.2us/block), so the emit
order skews L1 one block ahead and orders L2 z-chain first:
  PE queue : L1(k), L2(k-1)   |  ACT queue: z(k-1), F(k-1), h1(k)
which lets each engine run back-to-back instead of ping-ponging.

Other structure (from v4):
  - bf16 input [B, HH, C, 2, T, 2, W]; one 16KB-row DMA per h-quad.
  - L2 bwd-channel matmuls read h1 through time-REVERSED rhs APs so all
    post-L2 stages are direction-uniform: ONE 128-partition scan per pair.
    Host un-reverses bwd time at unshard.
  - z weights/bias negated -> ACT emits -z; g = (F-1)(-z) = (1-F)z in one
    scalar_tensor_tensor. Gate a = F from Sigmoid (same ACT table as Tanh);
    one memset zeroes segment starts.
  - PSUM: pH x2 + pZ x2 + pF x2 = 8 banks, all double-buffered.
  - bf16 output in scan layout; one DMA per h-quad on the Pool queue.
"""

import numpy as np

B, C, T, H, W = 2, 64, 32, 64, 64
NCORES = 8
HL = H // NCORES          # 8 h-rows per core
HP = HL // 2              # 4 h-pair tiles per batch entry
HH = HP // 2              # 2 h-quads per batch entry
S = 2 * W                 # 128 positions per h-pair (h2, w)
NJ = 4                    # 4 block-pairs per h-pair
CH = C // 2               # 32 = fwd (or bwd) channel count

_built = {}


def _build(repeat=1, order="h1_first", h1pair=False, quad_dma=False):
    """Build the kernel. repeat>1 emits the whole body that many times
    back-to-back in one NEFF (used by the benchmark to amortize per-launch
    host/runtime overhead out of the per-exec device-time measurement; each
    repeat re-does the full x-load/compute/y-store). order selects the
    ACT-queue emission order per pipeline iteration:
      "h1_first": h1(k) then z(k-1), F(k-1)
      "zf_first": z(k-1), F(k-1) then h1(k)"""
    import concourse.bass as bass
    import concourse.mybir as mybir
    import concourse.tile as tile
    from concourse import bacc

    fp32 = mybir.dt.float32
    bf16 = mybir.dt.bfloat16
    nc = bacc.Bacc(None, target_bir_lowering=False)

    x = nc.dram_tensor(
        "x", [B, HH, C, 2, NJ, 2, 16, T], bf16, kind="ExternalInput"
    )
    wall = nc.dram_tensor("wall", [3 * C, 2 * C], bf16, kind="ExternalInput")
    y = nc.dram_tensor(
        "y", [B, HH, 2 * C, 2, NJ * 512], bf16, kind="ExternalOutput"
    )

    MUL = mybir.AluOpType.mult
    ADD = mybir.AluOpType.add
    SUB = mybir.AluOpType.subtract
    TANH = mybir.ActivationFunctionType.Tanh
    SIGM = mybir.ActivationFunctionType.Sigmoid

    def rev_t(ap2d, tlen):
        # [P, F] contiguous AP -> [P, seg, t] with innermost t reversed.
        (pstep, pcnt), (fstep, fcnt) = [list(d) for d in ap2d.ap]
        assert fstep == 1 and fcnt % tlen == 0, ap2d.ap
        return bass.AP(
            tensor=ap2d.tensor,
            offset=ap2d.offset + (tlen - 1),
            ap=[[pstep, pcnt], [tlen, fcnt // tlen], [-1, tlen]],
        )

    with tile.TileContext(nc) as tc:
        with (
            tc.tile_pool(name="consts", bufs=1) as consts,
            tc.tile_pool(name="xin", bufs=4) as xin,
            tc.tile_pool(name="h1p", bufs=6) as h1p,
            tc.tile_pool(name="ew", bufs=5) as ew,
            tc.tile_pool(name="yout", bufs=3) as yout,
            tc.tile_pool(name="psH", bufs=(1 if h1pair else 2),
                         space="PSUM") as psH,
            tc.tile_pool(name="psZ", bufs=2, space="PSUM") as psZ,
            tc.tile_pool(name="psF", bufs=2, space="PSUM") as psF,
        ):
            # consts first on SP: tiny transfers, done before the first
            # (small, per-block) x slice lands.
            w1_sb = consts.tile([C, 2 * C], bf16)
            w2b_sb = consts.tile([2 * C, C + 3], bf16)
            nc.sync.dma_start(out=w1_sb, in_=wall[0:C, :])
            nc.sync.dma_start(out=w2b_sb, in_=wall[C : 3 * C, 0 : C + 3])
            w2_sb = w2b_sb[:, 0:C]
            # dummy sigmoid pins the tanh+sigmoid table load into warmup;
            # fed from a memset tile so it launches immediately.
            warm = consts.tile([2 * C, 2], bf16)
            nc.vector.memset(warm[:, 0:1], 0.0)
            nc.scalar.activation(warm[:, 1:2], warm[:, 0:1], SIGM)
            b_sb = consts.tile([2 * C, 3], fp32)
            nc.vector.tensor_copy(b_sb[:, :], w2b_sb[:, C : C + 3])

            # flat block list: (b, hh, u, jj, j2); 512 positions each
            blocks = [
                (b, hh, u, jj, j2)
                for b in range(B)
                for hh in range(HH)
                for u in range(2)
                for jj in range(NJ // 2)
                for j2 in range(2)
            ]
            N = len(blocks)
            quads = {}   # (b, hh, u) -> dict(x_t, y_t)
            st = {}      # k -> per-block tiles
            pairs = {}   # (b, hh, u, jj) -> dict(z, a, g)

            def emit_L1(k):
                b, hh, u, jj, j2 = blocks[k]
                qd = quads.get((b, hh, u))
                if qd is None:
                    x_t = xin.tile([C, NJ, 2, 16, T], bf16, name="x_t")
                    y_t = yout.tile([2 * C, NJ, 512], bf16, tag="yt", name="y_t")
                    qd = quads[(b, hh, u)] = dict(x_t=x_t, y_t=y_t)
                j = 2 * jj + j2
                if quad_dma:
                    # one [64 rows, 8KB] DMA per h-quad at its first block
                    if j == 0:
                        nc.sync.dma_start(
                            out=qd["x_t"].rearrange("c j r w t -> c (j r w t)"),
                            in_=x[b, hh, :, u].rearrange(
                                "c j r w t -> c (j r w t)"),
                        )
                else:
                    # per-block x slice: [64 rows, 2KB] contiguous both sides
                    nc.sync.dma_start(
                        out=qd["x_t"][:, j].rearrange("c r w t -> c (r w t)"),
                        in_=x[b, hh, :, u, j].rearrange("c r w t -> c (r w t)"),
                    )
                if h1pair:
                    if j2 == 0:
                        pH = psH.tile([2 * C, 4, 512], fp32)
                        st[k] = dict(pH=pH)
                    else:
                        pH = st[k - 1]["pH"]
                        st[k] = dict(pHref=pH)
                else:
                    pH = psH.tile([2 * C, 2, 512], fp32)
                    st[k] = dict(pH=pH)
                for q in range(2):
                    rhs = qd["x_t"][:, j, q]
                    nc.tensor.matmul(
                        pH[:, 2 * j2 + q if h1pair else q, :],
                        w1_sb[:, :], rhs, start=True, stop=True,
                    )

            def emit_h1(k):
                if h1pair:
                    # one [128, 2048] tanh per block-pair, fired at j2 == 1
                    b, hh, u, jj, j2 = blocks[k]
                    if j2 == 0:
                        return
                    pH = st[k].pop("pHref")
                    st[k - 1].pop("pH")
                    h1 = h1p.tile([2 * C, 4, 512], bf16)
                    nc.scalar.activation(
                        h1.rearrange("p a n -> p (a n)"),
                        pH.rearrange("p a n -> p (a n)"),
                        TANH, bias=b_sb[:, 0:1],
                    )
                    st[k - 1]["h1"] = h1[:, 0:2]
                    st[k]["h1"] = h1[:, 2:4]
                    return
                pH = st[k].pop("pH")
                h1 = h1p.tile([2 * C, 2, 512], bf16)
                nc.scalar.activation(
                    h1.rearrange("p a n -> p (a n)"),
                    pH.rearrange("p a n -> p (a n)"),
                    TANH, bias=b_sb[:, 0:1],
                )
                st[k]["h1"] = h1

            def emit_L2(k):
                h1 = st[k].pop("h1")
                pZ = psZ.tile([2 * C, 512], fp32)
                pF = psF.tile([2 * C, 512], fp32)
                # z-chain first so the z activation unblocks at 50% of L2
                for q in range(2):
                    col = 64 * q
                    hz = h1[0:C, q, :]
                    nc.tensor.matmul(
                        pZ[col : col + CH, :], w2_sb[0:C, 0:CH], hz,
                        start=True, stop=True, tile_position=(0, col),
                    )
                    nc.tensor.matmul(
                        pZ[col + CH : col + C, :], w2_sb[0:C, CH:C],
                        rev_t(hz, T),
                        start=True, stop=True, tile_position=(0, col + CH),
                    )
                for q in range(2):
                    col = 64 * q
                    hf = h1[C : 2 * C, q, :]
                    nc.tensor.matmul(
                        pF[col : col + CH, :], w2_sb[C : 2 * C, 0:CH], hf,
                        start=True, stop=True, tile_position=(64, col),
                    )
                    nc.tensor.matmul(
                        pF[col + CH : col + C, :], w2_sb[C : 2 * C, CH:C],
                        rev_t(hf, T),
                        start=True, stop=True, tile_position=(64, col + CH),
                    )
                st[k]["pZ"] = pZ
                st[k]["pF"] = pF

            def emit_zf(k):
                b, hh, u, jj, j2 = blocks[k]
                pr = pairs.get((b, hh, u, jj))
                if pr is None:
                    pr = pairs[(b, hh, u, jj)] = dict(
                        z=ew.tile([2 * C, 2, 16, 32], bf16, tag="z", name="z_sb"),
                        g=ew.tile([2 * C, 2, 16, 32], bf16, tag="g", name="g_sb"),
                        a=[None, None],
                    )
                pZ = st[k].pop("pZ")
                pF = st[k].pop("pF")
                nc.scalar.activation(
                    pr["z"][:, j2].rearrange("p s t -> p (s t)"),
                    pZ[:, :], TANH, bias=b_sb[:, 1:2],
                )
                # gate F written back in place: PSUM access is cheaper for ACT
                nc.scalar.activation(
                    pF[:, :], pF[:, :], SIGM, bias=b_sb[:, 2:3],
                )
                pr["a"][j2] = pF
                del st[k]

            def emit_epi(k, final=False):
                # per-block gate+scan; gate a lives in pF (PSUM, in-place
                # sigmoid), z in SBUF -- stt src constraint satisfied.
                b, hh, u, jj, j2 = blocks[k]
                pr = pairs.get((b, hh, u, jj))
                qd = quads[(b, hh, u)]
                sl = slice(j2, j2 + 1)
                z2 = pr["z"][:, sl].rearrange("p a s t -> p (a s t)")
                g2 = pr["g"][:, sl].rearrange("p a s t -> p (a s t)")
                aF = pr["a"][j2]
                a3 = aF.rearrange("p (s t) -> p s t", s=16)
                nc.vector.scalar_tensor_tensor(g2, aF[:, :], 1.0, z2, SUB, MUL)
                nc.vector.memset(a3[:, :, 0], 0.0)
                ysl = slice(2 * jj + j2, 2 * jj + j2 + 1)
                yj = qd["y_t"][:, ysl].rearrange("p j n -> p (j n)")
                nc.vector.tensor_tensor_scan(yj, aF[:, :], g2, 0.0, MUL, ADD)
                if j2 != 1:
                    return
                pairs.pop((b, hh, u, jj))
                last_tile = (b, hh, u) == blocks[-1][:3]
                if last_tile and jj == 0:
                    # last tile: ship the first half as soon as it's done
                    nc.gpsimd.dma_start(
                        out=y[b, hh, :, u, 0 : NJ * 256],
                        in_=qd["y_t"][:, 0 : NJ // 2].rearrange(
                            "p j n -> p (j n)"
                        ),
                    )
                elif last_tile and jj == NJ // 2 - 1:
                    # ship the last pair per block: the j=2 slice overlaps
                    # the final block's gate+scan; the tail DMA is half-size
                    y_t = qd.pop("y_t")
                    nc.gpsimd.dma_start(
                        out=y[b, hh, :, u, NJ * 256 : NJ * 384],
                        in_=y_t[:, 2:3].rearrange("p j n -> p (j n)"),
                    )
                    nc.gpsimd.dma_start(
                        out=y[b, hh, :, u, NJ * 384 :],
                        in_=y_t[:, 3:4].rearrange("p j n -> p (j n)"),
                    )
                    del quads[(b, hh, u)]
                elif jj == NJ // 2 - 1:
                    nc.gpsimd.dma_start(
                        out=y[b, hh, :, u],
                        in_=qd.pop("y_t").rearrange("p j n -> p (j n)"),
                    )
                    del quads[(b, hh, u)]

            # software-pipelined emission (skew 1):
            #   PE : L1(k), L2(k-1) | ACT: z(k-1), F(k-1), h1(k)
            for _rep in range(repeat):
                emit_L1(0)
                emit_h1(0)
                for k in range(1, N):
                    emit_L1(k)
                    if h1pair:
                        # pair-act must precede L2 of the pair's 1st block
                        emit_h1(k)
                        emit_L2(k - 1)
                        emit_zf(k - 1)
                    elif order == "zf_first":
                        emit_L2(k - 1)
                        emit_zf(k - 1)
                        emit_h1(k)
                    else:
                        emit_L2(k - 1)
                        emit_h1(k)
                        emit_zf(k - 1)
                    emit_epi(k - 1, final=(k - 1 == N - 2))
                emit_L2(N - 1)
                emit_zf(N - 1)
                emit_epi(N - 1, final=True)
    nc.compile()
    return nc


def _prep_weights(wz1, bz1, wz2, bz2, wf1, bf1, wf2, bf2):
    import ml_dtypes

    f32 = np.float32
    wall = np.zeros((3 * C, 2 * C), dtype=f32)
    wall[0:C, :] = np.concatenate([wz1, wf1], axis=0).T  # L1 lhsT
    wall[C : 2 * C, 0:C] = -wz2.T                        # negated: tanh -> -z
    wall[2 * C : 3 * C, 0:C] = wf2.T
    # biases ride in an unused corner of wall (bf16; converted on-device)
    wall[C : 3 * C, C : C + 3] = np.stack(
        [
            np.concatenate([bz1, bf1]),
            np.concatenate([-bz2, -bz2]),
            np.concatenate([bf2, bf2]),
        ],
        axis=1,
    )
    wall = wall.astype(ml_dtypes.bfloat16)
    return dict(wall=wall)


def _prep_x(xin):
    """Full fp32 input -> per-core bf16 [B, HH, C, u, j, r, w16, T] shards:
    each 512-position block is a 2KB-contiguous run per channel row."""
    import ml_dtypes

    xb = np.asarray(xin, dtype=np.float32).astype(ml_dtypes.bfloat16)
    xr = xb.reshape(B, C, T, NCORES, HH, 2, 2, NJ, 16)
    return [
        np.ascontiguousarray(
            xr[:, :, :, core].transpose(0, 3, 1, 4, 6, 5, 7, 2)
        )
        for core in range(NCORES)
    ]


def _unshard_y(results):
    """Per-core bf16 [B, HH, 128, 2, 2048] -> full fp32 [B, C, T, H, W].

    Row r = 64q + 32d + c (q = h-row in pair, d = direction); free
    (u = h-pair in quad, j, s, t); bwd rows (d=1) carry time reversed.
    """
    outs = []
    for r in results:
        buf = np.asarray(r["y"], dtype=np.float32).reshape(
            B, HH, 2, 2, CH, 2, NJ, 16, T
        )
        fwd = buf[:, :, :, 0]
        bwd = buf[:, :, :, 1, :, :, :, :, ::-1]
        st = np.stack([fwd, bwd], axis=3)  # [B, HH, q, d, c, u, j, s, t]
        o = st.transpose(0, 3, 4, 8, 1, 5, 2, 6, 7).reshape(B, C, T, HL, W)
        outs.append(o)
    return np.concatenate(outs, axis=3)


def kernel(inputs, wz1, bz1, wz2, bz2, wf1, bf1, wf2, bf2):
    from concourse.bass_utils import run_bass_kernel_spmd

    if "nc" not in _built:
        _built["nc"] = _build()
    nc = _built["nc"]

    wd = _prep_weights(
        np.asarray(wz1), np.asarray(bz1), np.asarray(wz2), np.asarray(bz2),
        np.asarray(wf1), np.asarray(bf1), np.asarray(wf2), np.asarray(bf2),
    )
    in_maps = []
    for shard in _prep_x(inputs):
        m = {"x": shard}
        m.update(wd)
        in_maps.append(m)

    res = run_bass_kernel_spmd(nc, in_maps, core_ids=list(range(NCORES)))
    return _unshard_y(res.results)



# revision 24
# speedup vs baseline: 1.1321x; 1.0057x over previous
"""Trainium2 Bass kernel for bidirectional gated linear recurrence block, v7.

v7 = v6 + deeper SBUF tile pools (xin 4, h1p 6, ew 5, yout 3 -- extra
scheduling slack at zero sim cost) and a `repeat` build parameter that
emits the body N times in one NEFF so the benchmark can amortize the
multi-hundred-microsecond per-launch host/tunnel dispatch cost out of
the per-exec device-time measurement. Cost-model engine occupancy per
exec: ACT 74us (critical, ~100%% busy in steady state), PE 68us, DVE
41us, DMA 39us; CoreSim end-to-end 79.7us (71.4us/rep at repeat=24 --
cross-rep pipelining hides warmup/drain), HW-measured ~70.1-70.6us/exec
(128x-amortized lower-quartile slope). Rejected by experiment: zf-first ACT
order (+0.5us), pair-merged h1 act on single-buffered 4-bank pH
(+32us -- L1 stalls on the pair act), SBUF-bf16 gate for the scan
(+0.9us -- ACT pays SBUF access on the F act).


Reference computation (per spatial position, channel-mixing MLPs):
  Z = tanh(W_z2 @ tanh(W_z1 @ x + b_z1) + b_z2)
  F = sigmoid(W_f2 @ tanh(W_f1 @ x + b_f1) + b_f2)
  channels 0:32  : h_t = F*h_{t-1} + (1-F)*Z forward over T
  channels 32:64 : same recurrence backward over T

Sharding: H (=64) split across 8 cores, 8 rows each; no collectives.

v6 = v5 + warmup/drain trims: per-h-pair x/y DMAs, segment-start memset
on DVE, and a dummy sigmoid up front so the combined tanh+sigmoid ACT
table loads once during warmup.

v5 core: explicit software pipelining. Per 512-position block k the work is
  PE : L1(k) 2 matmuls (427ns) + L2(k) 8 narrow matmuls (1707ns)
  ACT: h1(k) tanh [128,1024] (997ns) + z(k)/F(k) [128,512] (612ns each)
and the two engines have near-equal totals (~2.2us/block), so the emit
order skews L1 one block ahead and orders L2 z-chain first:
  PE queue : L1(k), L2(k-1)   |  ACT queue: z(k-1), F(k-1), h1(k)
which lets each engine run back-to-back instead of ping-ponging.

Other structure (from v4):
  - bf16 input [B, HH, C, 2, T, 2, W]; one 16KB-row DMA per h-quad.
  - L2 bwd-channel matmuls read h1 through time-REVERSED rhs APs so all
    post-L2 stages are direction-uniform: ONE 128-partition scan per pair.
    Host un-reverses bwd time at unshard.
  - z weights/bias negated -> ACT emits -z; g = (F-1)(-z) = (1-F)z in one
    scalar_tensor_tensor. Gate a = F from Sigmoid (same ACT table as Tanh);
    one memset zeroes segment starts.
  - PSUM: pH x2 + pZ x2 + pF x2 = 8 banks, all double-buffered.
  - bf16 output in scan layout; one DMA per h-quad on the Pool queue.
"""

import numpy as np

B, C, T, H, W = 2, 64, 32, 64, 64
NCORES = 8
HL = H // NCORES          # 8 h-rows per core
HP = HL // 2              # 4 h-pair tiles per batch entry
HH = HP // 2              # 2 h-quads per batch entry
S = 2 * W                 # 128 positions per h-pair (h2, w)
NJ = 4                    # 4 block-pairs per h-pair
CH = C // 2               # 32 = fwd (or bwd) channel count

_built = {}


def _build(repeat=1, order="h1_first", h1pair=False, quad_dma=False, zpair=False):
    """Build the kernel. repeat>1 emits the whole body that many times
    back-to-back in one NEFF (used by the benchmark to amortize per-launch
    host/runtime overhead out of the per-exec device-time measurement; each
    repeat re-does the full x-load/compute/y-store). order selects the
    ACT-queue emission order per pipeline iteration:
      "h1_first": h1(k) then z(k-1), F(k-1)
      "zf_first": z(k-1), F(k-1) then h1(k)"""
    import concourse.bass as bass
    import concourse.mybir as mybir
    import concourse.tile as tile
    from concourse import bacc

    fp32 = mybir.dt.float32
    bf16 = mybir.dt.bfloat16
    nc = bacc.Bacc(None, target_bir_lowering=False)

    x = nc.dram_tensor(
        "x", [B, HH, C, 2, NJ, 2, 16, T], bf16, kind="ExternalInput"
    )
    wall = nc.dram_tensor("wall", [3 * C, 2 * C], bf16, kind="ExternalInput")
    y = nc.dram_tensor(
        "y", [B, HH, 2 * C, 2, NJ * 512], bf16, kind="ExternalOutput"
    )

    MUL = mybir.AluOpType.mult
    ADD = mybir.AluOpType.add
    SUB = mybir.AluOpType.subtract
    TANH = mybir.ActivationFunctionType.Tanh
    SIGM = mybir.ActivationFunctionType.Sigmoid

    def rev_t(ap2d, tlen):
        # [P, F] contiguous AP -> [P, seg, t] with innermost t reversed.
        (pstep, pcnt), (fstep, fcnt) = [list(d) for d in ap2d.ap]
        assert fstep == 1 and fcnt % tlen == 0, ap2d.ap
        return bass.AP(
            tensor=ap2d.tensor,
            offset=ap2d.offset + (tlen - 1),
            ap=[[pstep, pcnt], [tlen, fcnt // tlen], [-1, tlen]],
        )

    with tile.TileContext(nc) as tc:
        with (
            tc.tile_pool(name="consts", bufs=1) as consts,
            tc.tile_pool(name="xin", bufs=4) as xin,
            tc.tile_pool(name="h1p", bufs=6) as h1p,
            tc.tile_pool(name="ew", bufs=5) as ew,
            tc.tile_pool(name="yout", bufs=3) as yout,
            tc.tile_pool(name="psH", bufs=(1 if h1pair else 2),
                         space="PSUM") as psH,
            tc.tile_pool(name="psZ", bufs=(1 if zpair else 2),
                         space="PSUM") as psZ,
            tc.tile_pool(name="psF", bufs=2, space="PSUM") as psF,
        ):
            # consts first on SP: tiny transfers, done before the first
            # (small, per-block) x slice lands.
            w1_sb = consts.tile([C, 2 * C], bf16)
            w2b_sb = consts.tile([2 * C, C + 3], bf16)
            nc.sync.dma_start(out=w1_sb, in_=wall[0:C, :])
            nc.sync.dma_start(out=w2b_sb, in_=wall[C : 3 * C, 0 : C + 3])
            w2_sb = w2b_sb[:, 0:C]
            # dummy sigmoid pins the tanh+sigmoid table load into warmup;
            # fed from a memset tile so it launches immediately.
            warm = consts.tile([2 * C, 2], bf16)
            nc.vector.memset(warm[:, 0:1], 0.0)
            nc.scalar.activation(warm[:, 1:2], warm[:, 0:1], SIGM)
            b_sb = consts.tile([2 * C, 3], fp32)
            nc.vector.tensor_copy(b_sb[:, :], w2b_sb[:, C : C + 3])

            # flat block list: (b, hh, u, jj, j2); 512 positions each
            blocks = [
                (b, hh, u, jj, j2)
                for b in range(B)
                for hh in range(HH)
                for u in range(2)
                for jj in range(NJ // 2)
                for j2 in range(2)
            ]
            N = len(blocks)
            quads = {}   # (b, hh, u) -> dict(x_t, y_t)
            st = {}      # k -> per-block tiles
            pairs = {}   # (b, hh, u, jj) -> dict(z, a, g)

            def emit_L1(k):
                b, hh, u, jj, j2 = blocks[k]
                qd = quads.get((b, hh, u))
                if qd is None:
                    x_t = xin.tile([C, NJ, 2, 16, T], bf16, name="x_t")
                    y_t = yout.tile([2 * C, NJ, 512], bf16, tag="yt", name="y_t")
                    qd = quads[(b, hh, u)] = dict(x_t=x_t, y_t=y_t)
                j = 2 * jj + j2
                if quad_dma:
                    # one [64 rows, 8KB] DMA per h-quad at its first block
                    if j == 0:
                        nc.sync.dma_start(
                            out=qd["x_t"].rearrange("c j r w t -> c (j r w t)"),
                            in_=x[b, hh, :, u].rearrange(
                                "c j r w t -> c (j r w t)"),
                        )
                else:
                    # per-block x slice: [64 rows, 2KB] contiguous both sides
                    nc.sync.dma_start(
                        out=qd["x_t"][:, j].rearrange("c r w t -> c (r w t)"),
                        in_=x[b, hh, :, u, j].rearrange("c r w t -> c (r w t)"),
                    )
                if h1pair:
                    if j2 == 0:
                        pH = psH.tile([2 * C, 4, 512], fp32)
                        st[k] = dict(pH=pH)
                    else:
                        pH = st[k - 1]["pH"]
                        st[k] = dict(pHref=pH)
                else:
                    pH = psH.tile([2 * C, 2, 512], fp32)
                    st[k] = dict(pH=pH)
                for q in range(2):
                    rhs = qd["x_t"][:, j, q]
                    nc.tensor.matmul(
                        pH[:, 2 * j2 + q if h1pair else q, :],
                        w1_sb[:, :], rhs, start=True, stop=True,
                    )

            def emit_h1(k):
                if h1pair:
                    # one [128, 2048] tanh per block-pair, fired at j2 == 1
                    b, hh, u, jj, j2 = blocks[k]
                    if j2 == 0:
                        return
                    pH = st[k].pop("pHref")
                    st[k - 1].pop("pH")
                    h1 = h1p.tile([2 * C, 4, 512], bf16)
                    nc.scalar.activation(
                        h1.rearrange("p a n -> p (a n)"),
                        pH.rearrange("p a n -> p (a n)"),
                        TANH, bias=b_sb[:, 0:1],
                    )
                    st[k - 1]["h1"] = h1[:, 0:2]
                    st[k]["h1"] = h1[:, 2:4]
                    return
                pH = st[k].pop("pH")
                h1 = h1p.tile([2 * C, 2, 512], bf16)
                nc.scalar.activation(
                    h1.rearrange("p a n -> p (a n)"),
                    pH.rearrange("p a n -> p (a n)"),
                    TANH, bias=b_sb[:, 0:1],
                )
                st[k]["h1"] = h1

            def emit_L2(k):
                b, hh, u, jj, j2 = blocks[k]
                h1 = st[k].pop("h1")
                if zpair:
                    if j2 == 0:
                        pZp = psZ.tile([2 * C, 2, 512], fp32)
                        pairs[("pZp", b, hh, u, jj)] = pZp
                    else:
                        pZp = pairs[("pZp", b, hh, u, jj)]
                    st[k]["pZp"] = pZp
                    pZ = pZp[:, j2, :]
                else:
                    pZ = psZ.tile([2 * C, 512], fp32)
                pF = psF.tile([2 * C, 512], fp32)
                # z-chain first so the z activation unblocks at 50% of L2
                for q in range(2):
                    col = 64 * q
                    hz = h1[0:C, q, :]
                    nc.tensor.matmul(
                        pZ[col : col + CH, :], w2_sb[0:C, 0:CH], hz,
                        start=True, stop=True, tile_position=(0, col),
                    )
                    nc.tensor.matmul(
                        pZ[col + CH : col + C, :], w2_sb[0:C, CH:C],
                        rev_t(hz, T),
                        start=True, stop=True, tile_position=(0, col + CH),
                    )
                for q in range(2):
                    col = 64 * q
                    hf = h1[C : 2 * C, q, :]
                    nc.tensor.matmul(
                        pF[col : col + CH, :], w2_sb[C : 2 * C, 0:CH], hf,
                        start=True, stop=True, tile_position=(64, col),
                    )
                    nc.tensor.matmul(
                        pF[col + CH : col + C, :], w2_sb[C : 2 * C, CH:C],
                        rev_t(hf, T),
                        start=True, stop=True, tile_position=(64, col + CH),
                    )
                if not zpair:
                    st[k]["pZ"] = pZ
                st[k]["pF"] = pF

            def emit_zf(k):
                b, hh, u, jj, j2 = blocks[k]
                pr = pairs.get((b, hh, u, jj))
                if pr is None:
                    pr = pairs[(b, hh, u, jj)] = dict(
                        z=ew.tile([2 * C, 2, 16, 32], bf16, tag="z", name="z_sb"),
                        g=ew.tile([2 * C, 2, 16, 32], bf16, tag="g", name="g_sb"),
                        a=[None, None],
                    )
                pF = st[k].pop("pF")
                if zpair and j2 == 0:
                    st[k].pop("pZp", None)
                if zpair:
                    # one [128,1024] z-tanh per block-pair (same per-
                    # partition bias across both halves), emitted early in
                    # the pair-end ACT batch so next pair's z-mms unblock.
                    if j2 == 1:
                        pZp = st[k].pop("pZp")
                        pairs.pop(("pZp", b, hh, u, jj))
                        nc.scalar.activation(
                            pr["z"].rearrange("p a s t -> p (a s t)"),
                            pZp.rearrange("p a n -> p (a n)"),
                            TANH, bias=b_sb[:, 1:2],
                        )
                else:
                    pZ = st[k].pop("pZ")
                    nc.scalar.activation(
                        pr["z"][:, j2].rearrange("p s t -> p (s t)"),
                        pZ[:, :], TANH, bias=b_sb[:, 1:2],
                    )
                # gate F written back in place: PSUM access is cheaper for ACT
                nc.scalar.activation(
                    pF[:, :], pF[:, :], SIGM, bias=b_sb[:, 2:3],
                )
                pr["a"][j2] = pF
                del st[k]

            def emit_epi(k, final=False):
                # per-block gate+scan; gate a lives in pF (PSUM, in-place
                # sigmoid), z in SBUF -- stt src constraint satisfied.
                b, hh, u, jj, j2 = blocks[k]
                pr = pairs.get((b, hh, u, jj))
                qd = quads[(b, hh, u)]
                sl = slice(j2, j2 + 1)
                z2 = pr["z"][:, sl].rearrange("p a s t -> p (a s t)")
                g2 = pr["g"][:, sl].rearrange("p a s t -> p (a s t)")
                aF = pr["a"][j2]
                a3 = aF.rearrange("p (s t) -> p s t", s=16)
                nc.vector.scalar_tensor_tensor(g2, aF[:, :], 1.0, z2, SUB, MUL)
                nc.vector.memset(a3[:, :, 0], 0.0)
                ysl = slice(2 * jj + j2, 2 * jj + j2 + 1)
                yj = qd["y_t"][:, ysl].rearrange("p j n -> p (j n)")
                nc.vector.tensor_tensor_scan(yj, aF[:, :], g2, 0.0, MUL, ADD)
                if j2 != 1:
                    return
                pairs.pop((b, hh, u, jj))
                last_tile = (b, hh, u) == blocks[-1][:3]
                if last_tile and jj == 0:
                    # last tile: ship the first half as soon as it's done
                    nc.gpsimd.dma_start(
                        out=y[b, hh, :, u, 0 : NJ * 256],
                        in_=qd["y_t"][:, 0 : NJ // 2].rearrange(
                            "p j n -> p (j n)"
                        ),
                    )
                elif last_tile and jj == NJ // 2 - 1:
                    # ship the last pair per block: the j=2 slice overlaps
                    # the final block's gate+scan; the tail DMA is half-size
                    y_t = qd.pop("y_t")
                    nc.gpsimd.dma_start(
                        out=y[b, hh, :, u, NJ * 256 : NJ * 384],
                        in_=y_t[:, 2:3].rearrange("p j n -> p (j n)"),
                    )
                    nc.gpsimd.dma_start(
                        out=y[b, hh, :, u, NJ * 384 :],
                        in_=y_t[:, 3:4].rearrange("p j n -> p (j n)"),
                    )
                    del quads[(b, hh, u)]
                elif jj == NJ // 2 - 1:
                    nc.gpsimd.dma_start(
                        out=y[b, hh, :, u],
                        in_=qd.pop("y_t").rearrange("p j n -> p (j n)"),
                    )
                    del quads[(b, hh, u)]

            # software-pipelined emission (skew 1):
            #   PE : L1(k), L2(k-1) | ACT: z(k-1), F(k-1), h1(k)
            def emit_epis(km1, final=False):
                if not zpair:
                    emit_epi(km1, final=final)
                    return
                if blocks[km1][4] == 1:
                    emit_epi(km1 - 1)
                    emit_epi(km1, final=final)

            for _rep in range(repeat):
                emit_L1(0)
                emit_h1(0)
                for k in range(1, N):
                    emit_L1(k)
                    if h1pair:
                        # pair-act must precede L2 of the pair's 1st block
                        emit_h1(k)
                        emit_L2(k - 1)
                        emit_zf(k - 1)
                    elif order == "zf_first":
                        emit_L2(k - 1)
                        emit_zf(k - 1)
                        emit_h1(k)
                    else:
                        emit_L2(k - 1)
                        emit_h1(k)
                        emit_zf(k - 1)
                    emit_epis(k - 1, final=(k - 1 == N - 2))
                emit_L2(N - 1)
                emit_zf(N - 1)
                emit_epis(N - 1, final=True)
    nc.compile()
    return nc


def _prep_weights(wz1, bz1, wz2, bz2, wf1, bf1, wf2, bf2):
    import ml_dtypes

    f32 = np.float32
    wall = np.zeros((3 * C, 2 * C), dtype=f32)
    wall[0:C, :] = np.concatenate([wz1, wf1], axis=0).T  # L1 lhsT
    wall[C : 2 * C, 0:C] = -wz2.T                        # negated: tanh -> -z
    wall[2 * C : 3 * C, 0:C] = wf2.T
    # biases ride in an unused corner of wall (bf16; converted on-device)
    wall[C : 3 * C, C : C + 3] = np.stack(
        [
            np.concatenate([bz1, bf1]),
            np.concatenate([-bz2, -bz2]),
            np.concatenate([bf2, bf2]),
        ],
        axis=1,
    )
    wall = wall.astype(ml_dtypes.bfloat16)
    return dict(wall=wall)


def _prep_x(xin):
    """Full fp32 input -> per-core bf16 [B, HH, C, u, j, r, w16, T] shards:
    each 512-position block is a 2KB-contiguous run per channel row."""
    import ml_dtypes

    xb = np.asarray(xin, dtype=np.float32).astype(ml_dtypes.bfloat16)
    xr = xb.reshape(B, C, T, NCORES, HH, 2, 2, NJ, 16)
    return [
        np.ascontiguousarray(
            xr[:, :, :, core].transpose(0, 3, 1, 4, 6, 5, 7, 2)
        )
        for core in range(NCORES)
    ]


def _unshard_y(results):
    """Per-core bf16 [B, HH, 128, 2, 2048] -> full fp32 [B, C, T, H, W].

    Row r = 64q + 32d + c (q = h-row in pair, d = direction); free
    (u = h-pair in quad, j, s, t); bwd rows (d=1) carry time reversed.
    """
    outs = []
    for r in results:
        buf = np.asarray(r["y"], dtype=np.float32).reshape(
            B, HH, 2, 2, CH, 2, NJ, 16, T
        )
        fwd = buf[:, :, :, 0]
        bwd = buf[:, :, :, 1, :, :, :, :, ::-1]
        st = np.stack([fwd, bwd], axis=3)  # [B, HH, q, d, c, u, j, s, t]
        o = st.transpose(0, 3, 4, 8, 1, 5, 2, 6, 7).reshape(B, C, T, HL, W)
        outs.append(o)
    return np.concatenate(outs, axis=3)


def kernel(inputs, wz1, bz1, wz2, bz2, wf1, bf1, wf2, bf2):
    from concourse.bass_utils import run_bass_kernel_spmd

    if "nc" not in _built:
        _built["nc"] = _build()
    nc = _built["nc"]

    wd = _prep_weights(
        np.asarray(wz1), np.asarray(bz1), np.asarray(wz2), np.asarray(bz2),
        np.asarray(wf1), np.asarray(bf1), np.asarray(wf2), np.asarray(bf2),
    )
    in_maps = []
    for shard in _prep_x(inputs):
        m = {"x": shard}
        m.update(wd)
        in_maps.append(m)

    res = run_bass_kernel_spmd(nc, in_maps, core_ids=list(range(NCORES)))
    return _unshard_y(res.results)

